# revision 11
# baseline (speedup 1.0000x reference)
"""Trainium2 Bass kernel for the linear state rollout problem.

reference: s_{t+1} = A s_t + B u_t, resX[:, t, :] = s_t, s_0 = x[:, 0, :]
shapes: x [256, 1024, 64], u [256, 1024, 7], A [64, 64], B [64, 7]

Strategy (per core, data-parallel over batch: 32 batch elems / core):
  - Chunk the 1024-step scan into 32 chunks of K=32 steps.
  - Pass 1: stride-8 scan over all (chunk, batch) columns at once (zero
    initial state) -> per-chunk input response z_c.
  - Phase B: sequential scan over the 32 chunk carries (tiny matmuls):
    S_{c+1} = A^32 S_c + z_c.
  - Pass 2: stride-8 scan re-run with correct initial states, emitting
    every intermediate state; each matmul packs 2 timesteps into a
    128-partition PSUM tile ([s_{t+2q+2}; s_{t+2q+1}]), 4 matmuls per
    8-timestep step (fp32r = full PE rate at N=512).
  SBUF layout: states live on partitions 0-63, u windows on 64-119
  (fp32r matmul outputs must land at PSUM partition 0).
  All A-power / weight matrices are computed on device from A and B.
  Host only does layout shuffles (transpose/reshape) and final assembly.
"""

import numpy as np

import concourse.bass as bass
import concourse.bacc as bacc
import concourse.tile as tile
from concourse import mybir
from concourse.bass_utils import run_bass_kernel_spmd

F32 = mybir.dt.float32
F32R = mybir.dt.float32r

NCORES = 8
BC = 32          # batch per core
T = 1024
L = 64
CH = 7
K = 32           # chunk length
NCHUNK = T // K  # 32
G = 2            # column groups
CPG = NCHUNK // G   # 16 chunks per group
NG = CPG * BC       # 512 columns per group
S = 4            # window steps per chunk (K / 8)
R = 8            # timesteps per window step
NQ = 4           # psum tiles per window step (2 timesteps each)

_NC_CACHE = None
LAST_RESULT = None


def _build_nc():
    nc = bacc.Bacc("TRN2", target_bir_lowering=False, debug=False,
                   num_devices=NCORES)

    uw = {}
    for s in range(S):
        for g in range(G):
            uw[s, g] = nc.dram_tensor(
                f"u_{s}_{g}", [64, NG], F32R, kind="ExternalInput").ap()
    s0 = nc.dram_tensor("s0", [L, BC], F32R, kind="ExternalInput").ap()
    # consts: [A | A^T | Bpad8 | I] column blocks
    consts = nc.dram_tensor("consts", [L, 2 * L + 8 + L], F32,
                            kind="ExternalInput").ap()
    identf = nc.dram_tensor("identf", [L, L], F32, kind="ExternalInput").ap()
    out = nc.dram_tensor(
        "out", [G * S * NQ, 128, NG], mybir.dt.float16,
        kind="ExternalOutput").ap()

    with tile.TileContext(nc) as tc:
        with (
            tc.tile_pool(name="const", bufs=1) as constp,
            tc.tile_pool(name="wts", bufs=1) as wp,
            tc.tile_pool(name="slabs", bufs=1) as slabp,
            tc.tile_pool(name="stage", bufs=1) as stagep,
            tc.tile_pool(name="prep_ps", bufs=3, space="PSUM") as prep_ps,
            tc.tile_pool(name="mm_ps", bufs=5, space="PSUM") as mm_ps,
        ):
            # ---- constants into SBUF (single DMA)
            const_sb = constp.tile([L, 2 * L + 8 + L], F32, tag="consts")
            nc.sync.dma_start(out=const_sb[:], in_=consts)
            identf_sb = constp.tile([L, L], F32, tag="identf")
            nc.sync.dma_start(out=identf_sb[:], in_=identf)
            amat_sb = const_sb[:, 0:L]
            atmat_sb = const_sb[:, L:2 * L]
            bmat_sb = const_sb[:, 2 * L:2 * L + 8]
            ident_sb = const_sb[:, 2 * L + 8:2 * L + 8 + L]

            zeros_sb = constp.tile([128, 128], F32, tag="zeros")
            nc.vector.memset(zeros_sb[:], 0.0)

            # ---- u windows into chain slabs rows 64-119 (+pad to 127)
            slab = {}
            for s in range(S):
                for g in range(G):
                    t_ = slabp.tile([128, NG], F32R, tag=f"slab{s}{g}",
                                    name=f"slab{s}{g}")
                    nc.sync.dma_start(out=t_[64:128, :], in_=uw[s, g])
                    slab[s, g] = t_
            # s0 -> chunk-0 initial state rows of slab (0, 0)
            nc.sync.dma_start(out=slab[0, 0][0:64, 0:BC], in_=s0)

            # ---- prep: P_p = (A^T)^p, R_p = A^p; depth-minimized chain
            def _pmm(lhsT, rhs, tag, dt_out=F32):
                ps = prep_ps.tile([64, L], F32, tag="prep", name="prep_ps_t")
                nc.tensor.matmul(ps[:, :], lhsT, rhs)
                t_ = wp.tile([64, L], dt_out, tag=tag, name=tag)
                nc.vector.tensor_copy(t_[:], ps[:])
                return t_

            P = {1: atmat_sb}
            Rr = {1: amat_sb}
            # L1
            P[2] = _pmm(amat_sb, P[1], "Pw2")
            Rr[2] = _pmm(atmat_sb, Rr[1], "Rw2")
            # L2
            P[3] = _pmm(amat_sb, P[2], "Pw3")
            P[4] = _pmm(Rr[2], P[2], "Pw4")
            Rr[3] = _pmm(atmat_sb, Rr[2], "Rw3")
            Rr[4] = _pmm(P[2], Rr[2], "Rw4")
            # L3
            P[5] = _pmm(amat_sb, P[4], "Pw5")
            P[6] = _pmm(Rr[2], P[4], "Pw6")
            P[7] = _pmm(Rr[3], P[4], "Pw7")
            P[8] = _pmm(Rr[4], P[4], "Pw8")
            Rr[8] = _pmm(P[4], Rr[4], "Rw8")
            # L4
            P16 = _pmm(Rr[8], P[8], "Pw16")
            R16 = _pmm(P[8], Rr[8], "Rw16")
            # L5
            P32 = _pmm(R16, P16, "Pw32")
            R32 = _pmm(P16, R16, "Rw32")
            # L6
            P64 = _pmm(R32, P32, "Pw64")
            R64 = _pmm(P32, R32, "Rw64")
            # L7
            P128 = _pmm(R64, P64, "Pw128")
            # fp32r-tagged copies for use as fp32r matmul weights
            P32r = wp.tile([64, L], F32R, tag="P32r")
            nc.vector.tensor_copy(P32r[:], P32[:, :])
            P64r = wp.tile([64, L], F32R, tag="P64r")
            nc.vector.tensor_copy(P64r[:], P64[:, :])
            P128r = wp.tile([64, L], F32R, tag="P128r")
            nc.vector.tensor_copy(P128r[:], P128[:, :])

            # ---- TBrev: (A^{7-j} B)^T at rows 8j..8j+6 (row 8j+7 zero)
            # 8-row blocks keep fp32r psum column offsets 8-aligned.
            tps = prep_ps.tile([64, 64], F32, tag="prep", name="prep_ps_t")
            for j in range(8):
                pw = 7 - j
                lhsT = ident_sb if pw == 0 else P[pw][:, :]
                nc.tensor.matmul(tps[0:64, 8 * j:8 * j + 8], lhsT,
                                 bmat_sb)
            tbt = wp.tile([64, 64], F32, tag="tbt")
            nc.vector.tensor_copy(tbt[:], tps[:])
            trp = prep_ps.tile([64, L], F32, tag="prep", name="prep_ps_t")
            nc.tensor.matmul(trp[0:64, 0:64], tbt[0:64, 0:64], identf_sb[:, :],
                             is_transpose=True)
            tbrev = wp.tile([64, L], F32R, tag="tbrev")
            nc.vector.tensor_copy(tbrev[:], trp[0:64, :])

            # ---- W1: rows 0-63 (A^8)^T, rows 64-127 u coeffs
            W1 = wp.tile([128, L], F32R, tag="W1")
            nc.vector.tensor_copy(W1[:], zeros_sb[:, 0:L])
            nc.vector.tensor_copy(W1[0:64, :], P[8][:, :])
            nc.sync.dma_start(out=W1[64:128, :], in_=tbrev[0:64, :])

            # ---- W2[q]: out cols 0-63 -> s_{t+2q+2}; 64-127 -> s_{t+2q+1}
            W2 = []
            for q in range(NQ):
                wt = wp.tile([128, 128], F32R, tag=f"W2{q}", name=f"W2{q}")
                nc.vector.tensor_copy(wt[:], zeros_sb[:])
                nc.vector.tensor_copy(wt[0:64, 0:64], P[2 * q + 2][:, :])
                nc.vector.tensor_copy(wt[0:64, 64:128], P[2 * q + 1][:, :])
                nhi = 8 * (2 * q + 2)
                nlo = 8 * (2 * q + 1)
                nc.sync.dma_start(out=wt[64:64 + nhi, 0:64],
                                  in_=tbrev[8 * (6 - 2 * q):8 * (6 - 2 * q) + nhi, :])
                nc.sync.dma_start(out=wt[64:64 + nlo, 64:128],
                                  in_=tbrev[8 * (7 - 2 * q):8 * (7 - 2 * q) + nlo, :])
                W2.append(wt)

            # ---- pass 1: z_c (zero-state chunk response), fp32r
            zt = {}
            for g in range(G):
                zt[g] = slabp.tile([128, NG], F32R, tag=f"z{g}",
                                   name=f"zt{g}")
            for s in range(S):
                for g in range(G):
                    zps = mm_ps.tile([128, NG], F32, tag="mmps", name="zps")
                    if s == 0:
                        nc.tensor.matmul(zps[0:64, :], W1[64:128, :],
                                         slab[0, g][64:128, :])
                    else:
                        nc.tensor.matmul(zps[0:64, :], W1[:, :],
                                         slab[s, g][:, :])
                    dst = slab[s + 1, g] if s < S - 1 else zt[g]
                    if (s + g) % 2 == 0:
                        nc.vector.tensor_copy(dst[0:64, :], zps[0:64, :])
                    else:
                        nc.scalar.copy(dst[0:64, :], zps[0:64, :])

            # ---- phase B precompute: w_c = A^K z_c + z_{c+1},
            #      v_c = A^{2K} w_c + w_{c+2}  (batched over chunk columns)
            w_sb = {}
            for g in range(G):
                w_sb[g] = slabp.tile([64, NG], F32R, tag=f"w{g}",
                                     name=f"wsb{g}")
                wps = mm_ps.tile([128, NG], F32, tag="mmps", name="wps")
                nc.tensor.matmul(wps[0:64, :], P32r[:, :], zt[g][0:64, :])
                nc.vector.tensor_add(w_sb[g][0:64, 0:NG - BC],
                                     wps[0:64, 0:NG - BC],
                                     zt[g][0:64, BC:NG])
                if g == 0:
                    nc.vector.tensor_add(w_sb[0][0:64, NG - BC:NG],
                                         wps[0:64, NG - BC:NG],
                                         zt[1][0:64, 0:BC])
            v_sb = {}
            for g in range(G):
                v_sb[g] = slabp.tile([64, NG], F32R, tag=f"v{g}",
                                     name=f"vsb{g}")
                vps = mm_ps.tile([128, NG], F32, tag="mmps", name="vps")
                nc.tensor.matmul(vps[0:64, :], P64r[:, :], w_sb[g][0:64, :])
                nc.vector.tensor_add(v_sb[g][0:64, 0:NG - 2 * BC],
                                     vps[0:64, 0:NG - 2 * BC],
                                     w_sb[g][0:64, 2 * BC:NG])
                if g == 0:
                    nc.vector.tensor_add(v_sb[0][0:64, NG - 2 * BC:NG],
                                         vps[0:64, NG - 2 * BC:NG],
                                         w_sb[1][0:64, 0:2 * BC])

            def _sslice(c):
                g, cl = divmod(c, CPG)
                return slab[0, g][0:64, cl * BC:(cl + 1) * BC]

            def _round(dst_c, src_c, pw, addend):
                bps = mm_ps.tile([64, BC], F32, tag="mmps", name="bps")
                nc.tensor.matmul(bps[:, :], pw[:, :], _sslice(src_c))
                nc.vector.tensor_add(_sslice(dst_c), bps[:, :], addend)

            def phase_b_round(c):
                # S_{c+4} = A^{4K} S_c + v_c
                g, cl = divmod(c, CPG)
                _round(c + 4, c, P128r, v_sb[g][0:64, cl * BC:(cl + 1) * BC])

            # init: S_1, S_2 from S_0; S_3 from S_1
            _round(1, 0, P32r, zt[0][0:64, 0:BC])
            _round(2, 0, P64r, w_sb[0][0:64, 0:BC])
            _round(3, 1, P64r, w_sb[0][0:64, BC:2 * BC])

            def pass2_step(g, m):
                # q order: 3 first so the carry chain advances earliest
                for q in (3, 0, 1, 2):
                    tau = g * S * NQ + m * NQ + q
                    qps = mm_ps.tile([128, NG], F32, tag="mmps", name="qps")
                    nc.tensor.matmul(qps[:, :], W2[q][:, :], slab[m, g][:, :])
                    st = stagep.tile([128, NG], mybir.dt.float16,
                                     tag=f"st{tau}", name=f"st{tau}")
                    if q == 0:
                        nc.vector.tensor_copy(st[:], qps[:])
                    else:
                        nc.scalar.copy(st[:], qps[:])
                    if q == NQ - 1 and m < S - 1:
                        # carry: next step's state, straight from PSUM
                        nc.vector.tensor_copy(slab[m + 1, g][0:64, :],
                                              qps[0:64, :])
                    nc.sync.dma_start(out=out[tau], in_=st[:])

            # rounds c=0..11 complete S_4..S_15 (group 0)
            for c in range(0, 12):
                phase_b_round(c)
            for m in range(S):
                pass2_step(0, m)
                for c in range(12 + 4 * m, min(12 + 4 * (m + 1), NCHUNK - 4)):
                    phase_b_round(c)
            for m in range(S):
                pass2_step(1, m)

    nc.compile()
    return nc


def _get_nc():
    global _NC_CACHE
    if _NC_CACHE is None:
        _NC_CACHE = _build_nc()
    return _NC_CACHE


def _build_in_maps(x, u, A, B):
    A = np.asarray(A, np.float32)
    B = np.asarray(B, np.float32)
    x = np.asarray(x, np.float32)
    u = np.asarray(u, np.float32)
    ident = np.eye(L, dtype=np.float32)
    bmat = np.ascontiguousarray(B)
    in_maps = []
    for core in range(NCORES):
        bsl = slice(core * BC, (core + 1) * BC)
        uc = u[bsl]                                  # [32, 1024, 7]
        w = uc.reshape(BC, G, CPG, S, R, CH)         # [b, g, cl, s, r, ch]
        m = {}
        for s in range(S):
            for g in range(G):
                win = w[:, g, :, s, :, :]            # [b, cl, r, ch]
                win = win.transpose(2, 3, 1, 0)      # [r, ch, cl, b]
                buf = np.zeros((R, 8, NG), np.float32)
                buf[:, :CH, :] = win.reshape(R, CH, NG)
                buf = buf.reshape(64, NG)
                m[f"u_{s}_{g}"] = buf
        m["s0"] = np.ascontiguousarray(x[bsl, 0, :].T)
        B8 = np.zeros((L, 8), np.float32)
        B8[:, :CH] = B
        m["consts"] = np.concatenate([A, A.T, B8, ident], axis=1)
        m["identf"] = ident
        in_maps.append(m)
    return in_maps


def kernel(x, u, A, B, stepNum):
    global LAST_RESULT
    stepNum = int(stepNum)
    nc = _get_nc()
    in_maps = _build_in_maps(x, u, A, B)
    res = run_bass_kernel_spmd(nc, in_maps, core_ids=list(range(NCORES)))
    LAST_RESULT = res
    out = np.empty((256, T, L), np.float32)
    for core in range(NCORES):
        od = np.asarray(res.results[core]["out"]).astype(np.float32)
        arr = od.reshape(G, S, NQ, 2, L, CPG, BC)    # [g, m, q, rr, l, cl, b]
        # rr=0 (partitions 0-63) holds t-offset 2q+2; rr=1 holds 2q+1.
        arr = arr[:, :, :, ::-1, :, :, :]            # flip rr -> r: 2q+1+r
        arr = arr.transpose(6, 0, 5, 1, 2, 3, 4)     # [b, g, cl, m, q, r, l]
        arr = np.ascontiguousarray(arr).reshape(BC, T, L)
        out[core * BC:(core + 1) * BC, 1:T, :] = arr[:, 0:T - 1, :]
    out[:, 0, :] = np.asarray(x, np.float32)[:, 0, :]
    if stepNum < T:
        out[:, stepNum:, :] = 0.0
    return out


# revision 12
# speedup vs baseline: 1.1415x; 1.1415x over previous
"""Trainium2 Bass kernel for the linear state rollout problem.

reference: s_{t+1} = A s_t + B u_t, resX[:, t, :] = s_t, s_0 = x[:, 0, :]
shapes: x [256, 1024, 64], u [256, 1024, 7], A [64, 64], B [64, 7]

Strategy (per core, data-parallel over batch: 32 batch elems / core):
  - Chunk the 1024-step scan into 32 chunks of K=32 steps.
  - Pass 1: stride-8 scan over all (chunk, batch) columns at once (zero
    initial state) -> per-chunk input response z_c.
  - Phase B: sequential scan over the 32 chunk carries (tiny matmuls):
    S_{c+1} = A^32 S_c + z_c.
  - Pass 2: stride-8 scan re-run with correct initial states, emitting
    every intermediate state; each matmul packs 2 timesteps into a
    128-partition PSUM tile ([s_{t+2q+2}; s_{t+2q+1}]), 4 matmuls per
    8-timestep step (fp32r = full PE rate at N=512).
  SBUF layout: states live on partitions 0-63, u windows on 64-119
  (fp32r matmul outputs must land at PSUM partition 0).
  All A-power / weight matrices are computed on device from A and B.
  Host only does layout shuffles (transpose/reshape) and final assembly.
"""

import numpy as np

import concourse.bass as bass
import concourse.bacc as bacc
import concourse.tile as tile
from concourse import mybir
from concourse.bass_utils import run_bass_kernel_spmd

F32 = mybir.dt.float32
F32R = mybir.dt.float32r

NCORES = 8
BC = 32          # batch per core
T = 1024
L = 64
CH = 7
K = 32           # chunk length
NCHUNK = T // K  # 32
G = 2            # column groups
CPG = NCHUNK // G   # 16 chunks per group
NG = CPG * BC       # 512 columns per group
S = 4            # window steps per chunk (K / 8)
R = 8            # timesteps per window step
NQ = 4           # psum tiles per window step (2 timesteps each)

_NC_CACHE = None
LAST_RESULT = None


def _build_nc():
    nc = bacc.Bacc("TRN2", target_bir_lowering=False, debug=False,
                   num_devices=NCORES)

    uw = {}
    for s in range(S):
        for g in range(G):
            uw[s, g] = nc.dram_tensor(
                f"u_{s}_{g}", [64, NG], F32R, kind="ExternalInput").ap()
    s0 = nc.dram_tensor("s0", [L, BC], F32R, kind="ExternalInput").ap()
    # consts: [A | A^T | Bpad8 | I] column blocks
    consts = nc.dram_tensor("consts", [L, 2 * L + 8 + L], F32,
                            kind="ExternalInput").ap()
    identf = nc.dram_tensor("identf", [L, L], F32, kind="ExternalInput").ap()
    out = nc.dram_tensor(
        "out", [G * S * NQ, 128, NG], mybir.dt.float16,
        kind="ExternalOutput").ap()

    with tile.TileContext(nc) as tc:
        with (
            tc.tile_pool(name="const", bufs=1) as constp,
            tc.tile_pool(name="wts", bufs=1) as wp,
            tc.tile_pool(name="slabs", bufs=1) as slabp,
            tc.tile_pool(name="stage", bufs=1) as stagep,
            tc.tile_pool(name="prep_ps", bufs=2, space="PSUM") as prep_ps,
            tc.tile_pool(name="mm_ps", bufs=6, space="PSUM") as mm_ps,
        ):
            # ---- constants into SBUF (single DMA)
            const_sb = constp.tile([L, 2 * L + 8 + L], F32, tag="consts")
            nc.sync.dma_start(out=const_sb[:], in_=consts)
            identf_sb = constp.tile([L, L], F32, tag="identf")
            nc.sync.dma_start(out=identf_sb[:], in_=identf)
            amat_sb = const_sb[:, 0:L]
            atmat_sb = const_sb[:, L:2 * L]
            bmat_sb = const_sb[:, 2 * L:2 * L + 8]
            ident_sb = const_sb[:, 2 * L + 8:2 * L + 8 + L]

            zeros_sb = constp.tile([128, 128], F32, tag="zeros")
            nc.vector.memset(zeros_sb[:], 0.0)

            # ---- u windows into chain slabs rows 64-119 (+pad to 127)
            slab = {}
            for s in range(S):
                for g in range(G):
                    t_ = slabp.tile([128, NG], F32R, tag=f"slab{s}{g}",
                                    name=f"slab{s}{g}")
                    nc.sync.dma_start(out=t_[64:128, :], in_=uw[s, g])
                    slab[s, g] = t_
            # s0 -> chunk-0 initial state rows of slab (0, 0)
            nc.sync.dma_start(out=slab[0, 0][0:64, 0:BC], in_=s0)

            # ---- prep: P_p = (A^T)^p, R_p = A^p; depth-minimized chain
            def _pmm(lhsT, rhs, tag, dt_out=F32):
                ps = prep_ps.tile([64, L], F32, tag="prep", name="prep_ps_t")
                nc.tensor.matmul(ps[:, :], lhsT, rhs)
                t_ = wp.tile([64, L], dt_out, tag=tag, name=tag)
                nc.vector.tensor_copy(t_[:], ps[:])
                return t_

            P = {1: atmat_sb}
            Rr = {1: amat_sb}
            # L1
            P[2] = _pmm(amat_sb, P[1], "Pw2")
            Rr[2] = _pmm(atmat_sb, Rr[1], "Rw2")
            # L2
            P[3] = _pmm(amat_sb, P[2], "Pw3")
            P[4] = _pmm(Rr[2], P[2], "Pw4")
            Rr[3] = _pmm(atmat_sb, Rr[2], "Rw3")
            Rr[4] = _pmm(P[2], Rr[2], "Rw4")
            # L3
            P[5] = _pmm(amat_sb, P[4], "Pw5")
            P[6] = _pmm(Rr[2], P[4], "Pw6")
            P[7] = _pmm(Rr[3], P[4], "Pw7")
            P[8] = _pmm(Rr[4], P[4], "Pw8")
            Rr[8] = _pmm(P[4], Rr[4], "Rw8")
            # L4
            P16 = _pmm(Rr[8], P[8], "Pw16")
            R16 = _pmm(P[8], Rr[8], "Rw16")
            # L5
            P32 = _pmm(R16, P16, "Pw32")
            R32 = _pmm(P16, R16, "Rw32")
            # L6
            P64 = _pmm(R32, P32, "Pw64")
            R64 = _pmm(P32, R32, "Rw64")
            # L7
            P128 = _pmm(R64, P64, "Pw128")
            # fp32r-tagged copies for use as fp32r matmul weights
            P32r = wp.tile([64, L], F32R, tag="P32r")
            nc.vector.tensor_copy(P32r[:], P32[:, :])
            P64r = wp.tile([64, L], F32R, tag="P64r")
            nc.vector.tensor_copy(P64r[:], P64[:, :])
            P128r = wp.tile([64, L], F32R, tag="P128r")
            nc.vector.tensor_copy(P128r[:], P128[:, :])

            # ---- TBrev: (A^{7-j} B)^T at rows 8j..8j+6 (row 8j+7 zero)
            # 8-row blocks keep fp32r psum column offsets 8-aligned.
            tps = prep_ps.tile([64, 64], F32, tag="prep", name="prep_ps_t")
            for j in range(8):
                pw = 7 - j
                lhsT = ident_sb if pw == 0 else P[pw][:, :]
                nc.tensor.matmul(tps[0:64, 8 * j:8 * j + 8], lhsT,
                                 bmat_sb)
            tbt = wp.tile([64, 64], F32, tag="tbt")
            nc.vector.tensor_copy(tbt[:], tps[:])
            trp = prep_ps.tile([64, L], F32, tag="prep", name="prep_ps_t")
            nc.tensor.matmul(trp[0:64, 0:64], tbt[0:64, 0:64], identf_sb[:, :],
                             is_transpose=True)
            tbrev = wp.tile([64, L], F32R, tag="tbrev")
            nc.vector.tensor_copy(tbrev[:], trp[0:64, :])

            # ---- W1: rows 0-63 (A^8)^T, rows 64-127 u coeffs
            W1 = wp.tile([128, L], F32R, tag="W1")
            nc.vector.tensor_copy(W1[:], zeros_sb[:, 0:L])
            nc.vector.tensor_copy(W1[0:64, :], P[8][:, :])
            nc.sync.dma_start(out=W1[64:128, :], in_=tbrev[0:64, :])

            # ---- W2[q]: out cols 0-63 -> s_{t+2q+2}; 64-127 -> s_{t+2q+1}
            W2 = []
            for q in range(NQ):
                wt = wp.tile([128, 128], F32R, tag=f"W2{q}", name=f"W2{q}")
                nc.vector.tensor_copy(wt[:], zeros_sb[:])
                nc.vector.tensor_copy(wt[0:64, 0:64], P[2 * q + 2][:, :])
                nc.vector.tensor_copy(wt[0:64, 64:128], P[2 * q + 1][:, :])
                nhi = 8 * (2 * q + 2)
                nlo = 8 * (2 * q + 1)
                nc.sync.dma_start(out=wt[64:64 + nhi, 0:64],
                                  in_=tbrev[8 * (6 - 2 * q):8 * (6 - 2 * q) + nhi, :])
                nc.sync.dma_start(out=wt[64:64 + nlo, 64:128],
                                  in_=tbrev[8 * (7 - 2 * q):8 * (7 - 2 * q) + nlo, :])
                W2.append(wt)

            # ---- pass 1: z_c (zero-state chunk response), fp32r
            zt = {}
            for g in range(G):
                zt[g] = slabp.tile([128, NG], F32R, tag=f"z{g}",
                                   name=f"zt{g}")
            for s in range(S):
                for g in range(G):
                    zps = mm_ps.tile([128, NG], F32, tag="mmps", name="zps")
                    if s == 0:
                        nc.tensor.matmul(zps[0:64, :], W1[64:128, :],
                                         slab[0, g][64:128, :])
                    else:
                        nc.tensor.matmul(zps[0:64, :], W1[:, :],
                                         slab[s, g][:, :])
                    dst = slab[s + 1, g] if s < S - 1 else zt[g]
                    if (s + g) % 2 == 0:
                        nc.vector.tensor_copy(dst[0:64, :], zps[0:64, :])
                    else:
                        nc.scalar.copy(dst[0:64, :], zps[0:64, :])

            # ---- phase B precompute: w_c = A^K z_c + z_{c+1},
            #      v_c = A^{2K} w_c + w_{c+2}  (batched over chunk columns)
            w_sb = {}
            for g in range(G):
                w_sb[g] = slabp.tile([64, NG], F32R, tag=f"w{g}",
                                     name=f"wsb{g}")
                wps = mm_ps.tile([128, NG], F32, tag="mmps", name="wps")
                nc.tensor.matmul(wps[0:64, :], P32r[:, :], zt[g][0:64, :])
                nc.vector.tensor_add(w_sb[g][0:64, 0:NG - BC],
                                     wps[0:64, 0:NG - BC],
                                     zt[g][0:64, BC:NG])
                if g == 0:
                    nc.vector.tensor_add(w_sb[0][0:64, NG - BC:NG],
                                         wps[0:64, NG - BC:NG],
                                         zt[1][0:64, 0:BC])
            v_sb = {}
            for g in range(G):
                v_sb[g] = slabp.tile([64, NG], F32R, tag=f"v{g}",
                                     name=f"vsb{g}")
                vps = mm_ps.tile([128, NG], F32, tag="mmps", name="vps")
                nc.tensor.matmul(vps[0:64, :], P64r[:, :], w_sb[g][0:64, :])
                nc.vector.tensor_add(v_sb[g][0:64, 0:NG - 2 * BC],
                                     vps[0:64, 0:NG - 2 * BC],
                                     w_sb[g][0:64, 2 * BC:NG])
                if g == 0:
                    nc.vector.tensor_add(v_sb[0][0:64, NG - 2 * BC:NG],
                                         vps[0:64, NG - 2 * BC:NG],
                                         w_sb[1][0:64, 0:2 * BC])

            def _sslice(c):
                g, cl = divmod(c, CPG)
                return slab[0, g][0:64, cl * BC:(cl + 1) * BC]

            def _round(dst_c, src_c, pw, addend):
                bps = mm_ps.tile([64, BC], F32, tag="mmps", name="bps")
                nc.tensor.matmul(bps[:, :], pw[:, :], _sslice(src_c))
                nc.vector.tensor_add(_sslice(dst_c), bps[:, :], addend)

            def phase_b_round(c):
                # S_{c+4} = A^{4K} S_c + v_c
                g, cl = divmod(c, CPG)
                _round(c + 4, c, P128r, v_sb[g][0:64, cl * BC:(cl + 1) * BC])

            # init: S_1, S_2 from S_0; S_3 from S_1
            _round(1, 0, P32r, zt[0][0:64, 0:BC])
            _round(2, 0, P64r, w_sb[0][0:64, 0:BC])
            _round(3, 1, P64r, w_sb[0][0:64, BC:2 * BC])

            def pass2_step(g, m):
                # q order: 3 first so the carry chain advances earliest
                for q in (3, 0, 1, 2):
                    tau = g * S * NQ + m * NQ + q
                    qps = mm_ps.tile([128, NG], F32, tag="mmps", name="qps")
                    nc.tensor.matmul(qps[:, :], W2[q][:, :], slab[m, g][:, :])
                    st = stagep.tile([128, NG], mybir.dt.float16,
                                     tag=f"st{tau}", name=f"st{tau}")
                    if q in (3, 1):
                        nc.scalar.copy(st[:], qps[:])
                    else:
                        nc.vector.tensor_copy(st[:], qps[:])
                    if q == NQ - 1 and m < S - 1:
                        # carry: next step's state, straight from PSUM
                        nc.vector.tensor_copy(slab[m + 1, g][0:64, :],
                                              qps[0:64, :])
                    nc.sync.dma_start(out=out[tau], in_=st[:])

            # rounds c=0..11 complete S_4..S_15 (group 0)
            for c in range(0, 12):
                phase_b_round(c)
            for m in range(S):
                pass2_step(0, m)
                for c in range(12 + 4 * m, min(12 + 4 * (m + 1), NCHUNK - 4)):
                    phase_b_round(c)
            for m in range(S):
                pass2_step(1, m)

    nc.compile()
    return nc


def _get_nc():
    global _NC_CACHE
    if _NC_CACHE is None:
        _NC_CACHE = _build_nc()
    return _NC_CACHE


def _build_in_maps(x, u, A, B):
    A = np.asarray(A, np.float32)
    B = np.asarray(B, np.float32)
    x = np.asarray(x, np.float32)
    u = np.asarray(u, np.float32)
    ident = np.eye(L, dtype=np.float32)
    bmat = np.ascontiguousarray(B)
    in_maps = []
    for core in range(NCORES):
        bsl = slice(core * BC, (core + 1) * BC)
        uc = u[bsl]                                  # [32, 1024, 7]
        w = uc.reshape(BC, G, CPG, S, R, CH)         # [b, g, cl, s, r, ch]
        m = {}
        for s in range(S):
            for g in range(G):
                win = w[:, g, :, s, :, :]            # [b, cl, r, ch]
                win = win.transpose(2, 3, 1, 0)      # [r, ch, cl, b]
                buf = np.zeros((R, 8, NG), np.float32)
                buf[:, :CH, :] = win.reshape(R, CH, NG)
                buf = buf.reshape(64, NG)
                m[f"u_{s}_{g}"] = buf
        m["s0"] = np.ascontiguousarray(x[bsl, 0, :].T)
        B8 = np.zeros((L, 8), np.float32)
        B8[:, :CH] = B
        m["consts"] = np.concatenate([A, A.T, B8, ident], axis=1)
        m["identf"] = ident
        in_maps.append(m)
    return in_maps


def kernel(x, u, A, B, stepNum):
    global LAST_RESULT
    stepNum = int(stepNum)
    nc = _get_nc()
    in_maps = _build_in_maps(x, u, A, B)
    res = run_bass_kernel_spmd(nc, in_maps, core_ids=list(range(NCORES)))
    LAST_RESULT = res
    out = np.empty((256, T, L), np.float32)
    for core in range(NCORES):
        od = np.asarray(res.results[core]["out"]).astype(np.float32)
        arr = od.reshape(G, S, NQ, 2, L, CPG, BC)    # [g, m, q, rr, l, cl, b]
        # rr=0 (partitions 0-63) holds t-offset 2q+2; rr=1 holds 2q+1.
        arr = arr[:, :, :, ::-1, :, :, :]            # flip rr -> r: 2q+1+r
        arr = arr.transpose(6, 0, 5, 1, 2, 3, 4)     # [b, g, cl, m, q, r, l]
        arr = np.ascontiguousarray(arr).reshape(BC, T, L)
        out[core * BC:(core + 1) * BC, 1:T, :] = arr[:, 0:T - 1, :]
    out[:, 0, :] = np.asarray(x, np.float32)[:, 0, :]
    if stepNum < T:
        out[:, stepNum:, :] = 0.0
    return out
